# revision 5
# baseline (speedup 1.0000x reference)
"""GCN aggregation (SpMM + linear) on 8 Trainium2 NeuronCores.

out = segment_sum(feature[adj_cols] * adj_vals, adj_rows) @ W.T

Scatter-add design (no PE one-hot, no per-tile DVE work):
- Destination rows sharded across cores (12500/core); edges grouped by
  (source window of 25000, dest class d%4, merge category). Per (dest,
  window, class) the k incident edges split into k//8 octs, quads, pairs
  and singles.
- dma_gather pulls one 256B bf16 source row per edge token (descriptor-
  count-bound: ~2.3ns/desc over 4 SWDGE queues, independent of order).
- DVE scales tokens by val (f32) and merge-trees same-dest groups with
  1-3 pairwise-add levels, cutting scatter descriptors ~2.6x.
- dma_scatter_add (SBUF-dest CCE) accumulates merged tokens into per-class
  f32 accumulators (tokens_per_rank=128: partition=dc%128, parity=(dc>>7)&1
  routes own/peer, group=dc>>8). The CCE loses updates when two in-flight
  descriptors share an address, so every chunk carries distinct dests and
  the host keeps a dest out of neighboring chunks of its class queue; the
  accumulators are raw (untracked) SBUF so scatters never serialize, with
  per-queue DMA semaphores ordering the finale.
- Scatters issue LAG chunks behind their gathers so their waits are
  satisfied before reaching the GpSimd engine head (all SWDGE instructions
  share that one engine).
- Finale: f32->bf16, PE transpose per 128-dest block via identity matmul,
  out_T = wt^T @ agg_T in 512-col chunks; host interleaves class outputs.
"""

import os
import sys
import types

import numpy as np
import ml_dtypes

# ------------------------------------------------------------- env plumbing
def _install_axon_ntff_shim():
    """bass_utils' axon trace path imports antenv.axon_hooks, which the
    container image lacks; wire it to the ctypes hook in trn_agent_boot."""
    if "antenv.axon_hooks" in sys.modules:
        return
    try:
        import trn_agent_boot.trn_boot as tb

        hook = tb._ntff_profile_via_ctypes("/opt/axon/libaxon_pjrt.so")
    except Exception:
        hook = None
    mod = types.ModuleType("antenv.axon_hooks")
    mod.get_axon_ntff_profile_hook = lambda: hook
    import antenv  # noqa: F401  (package must exist for submodule resolution)

    sys.modules["antenv.axon_hooks"] = mod


def _split_excess_waits(nc):
    """This walrus build allows at most ONE sync wait per instruction.
    Tile's scheduler freely attaches several; hoist the excess onto NoOp
    wait-carriers inserted just before the instruction (same engine, so
    engine program order preserves the blocking semantics)."""
    import bass_rust
    import concourse.mybir as mybir

    for f in nc.m.functions:
        for bb in f.blocks:
            new = []
            dirty = False
            for ins in bb.instructions:
                si = ins.sync_info
                if si is not None and len(si.on_wait) > 1:
                    waits = list(si.on_wait)
                    for k, w in enumerate(waits[:-1]):
                        nop = mybir.InstNoOp(
                            name=f"{ins.name}-pw{k}", ins=[], outs=[]
                        )
                        nop.engine = ins.engine
                        nop.sync_info = bass_rust.SyncInfo(
                            on_wait=[w], on_update=[]
                        )
                        new.append(nop)
                    si.on_wait = waits[-1:]
                    dirty = True
                new.append(ins)
            if dirty:
                bb.instructions = new


def _patch_bacc_compile():
    """Append the wait-splitter to Bacc.compile so it runs after every
    other lowering pass (walrus allows 1 sync wait per instruction)."""
    import concourse.bacc as bacc

    if getattr(bacc.Bacc, "_gcn_split_patched", False):
        return
    orig = bacc.Bacc.compile

    def _compile(self):
        orig(self)
        _split_excess_waits(self)

    bacc.Bacc.compile = _compile
    bacc.Bacc._gcn_split_patched = True


def _patch_tile_drain():
    """This walrus build rejects >1 sync wait on an InstDrain; split the
    Tile tail-drain's waits across multiple drain instructions."""
    import bass_rust
    import concourse.tile as tile
    from concourse.vector_clock import ScopedClock

    if getattr(tile.TileContext, "_gcn_drain_patched", False):
        return

    def _patched(self, tick_clock, wait_clock):
        nc = self.nc
        drain_inst = nc.sync.drain()
        wait_clock.add_sem_waits(
            drain_inst.ins, ScopedClock({None: tick_clock.global_clock})
        )
        si = drain_inst.ins.sync_info
        waits = list(si.on_wait)
        if len(waits) > 1:
            si.on_wait = waits[:1]
            for i in range(1, len(waits)):
                d2 = nc.sync.drain()
                d2.ins.sync_info = bass_rust.SyncInfo(
                    on_wait=waits[i : i + 1], on_update=[]
                )
        nc.all_engine_barrier()
        assert self.sems is not None
        popped = nc._tile_sem_poison_stack.pop()
        assert popped is self._sem_poison
        nc.clear_and_free_semaphores(list(self.sems.allocated().values()))
        nc.all_engine_barrier()

    tile.TileContext._drain_and_barrier = _patched
    tile.TileContext._gcn_drain_patched = True



N_NODES = 100000
N_EDGES = 1600000
EMB = 64
NC = 8
NPC = N_NODES // NC          # 12500 dest rows per core
NWIN = 4
WSZ = N_NODES // NWIN        # 25000 source rows per window
NCLS = 4                     # dest class = d % 4
NDC = NPC // NCLS            # 3125 class-local dests
NGRP = 13                    # dc capacity 3328 (spare zone for dummies)
DUMMY_DC = 3300              # block 25 (peer side, grp 12) — discarded
PAD = 128                    # feature row padded to 128 bf16 = 256B
CORE_IDS = list(range(NC))
CATS = (("oct", 8), ("quad", 4), ("pair", 2), ("sing", 1))
CHUNK_RAW = 1024             # raw tokens per full chunk

LAST_EXEC_NS = None

_BF16 = ml_dtypes.bfloat16


def _pack16(a):
    """[n] int16 -> [128, n/16] (token i at [i%16, i//16], replicated 8x)."""
    assert len(a) % 16 == 0
    return np.ascontiguousarray(np.tile(a.reshape(-1, 16).T, (8, 1)))


# ---------------------------------------------------------- host preprocess
def _preprocess(rows, cols, vals):
    core = rows // NPC
    d = rows - core * NPC
    cls_arr = d % NCLS
    dc_arr = d // NCLS
    w_arr = cols // WSZ
    src16 = (cols - w_arr * WSZ).astype(np.int16)

    # per (core, w, cls): dc-sorted edge ids + per-cat group counts
    ncat = np.zeros((NC, NWIN, NCLS, 4), np.int64)
    buckets = {}
    for c in range(NC):
        mc = core == c
        for wi in range(NWIN):
            mw = mc & (w_arr == wi)
            for cl in range(NCLS):
                m = mw & (cls_arr == cl)
                dcs = dc_arr[m]
                order = np.argsort(dcs, kind="stable")
                idx_e = np.nonzero(m)[0][order]
                dcs_s = dcs[order]
                buckets[(c, wi, cl)] = (idx_e, dcs_s)
                k = np.bincount(dcs_s, minlength=NDC)
                ncat[c, wi, cl, 0] = (k // 8).sum()
                r = k % 8
                ncat[c, wi, cl, 1] = (r // 4).sum()
                r = r % 4
                ncat[c, wi, cl, 2] = (r // 2).sum()
                ncat[c, wi, cl, 3] = (r % 2).sum()

    # shared chunk structure: max group count over cores per (w, cls, cat).
    # Classes are interleaved in emission order so that consecutive scatters
    # on one queue always have >= 1 full gather chunk between them (race
    # spacing on the ring).
    chunks = []
    raw_off = 0
    mrg_off = 0
    for wi in range(NWIN):
        for ci, (cat, mult) in enumerate(CATS):
            per_chunk = CHUNK_RAW // mult
            percls = []
            for cl in range(NCLS):
                if ci == 0:
                    # full oct chunks only: a partial oct chunk reserves a
                    # whole 1024-raw window for few groups; the excess octs
                    # demote to quads in place() instead
                    n = (int(ncat[:, wi, cl, 0].max()) // 128) * 128
                elif ci == 1:
                    noct = ncat[:, wi, cl, 0]
                    cap = (int(noct.max()) // 128) * 128
                    n = int(
                        (ncat[:, wi, cl, 1] + 2 * np.maximum(0, noct - cap)).max()
                    )
                else:
                    n = int(ncat[:, wi, cl, ci].max())
                lst = []
                pos = 0
                while pos < n:
                    g = min(per_chunk, n - pos)
                    lst.append((cl, g, -(-g // 128)))
                    pos += g
                percls.append(lst)
            j = 0
            while any(percls):
                for cl in range(NCLS):
                    if percls[cl]:
                        cl_, g, mcols = percls[cl].pop(0)
                        chunks.append(
                            dict(
                                win=wi, cls=cl_, cat=ci, mult=mult,
                                n_groups=g, mcols=mcols,
                                raw_off=raw_off, mrg_off=mrg_off,
                            )
                        )
                        raw_off += mcols * mult * 128
                        mrg_off += mcols * 128
    raw_total = raw_off
    mrg_total = mrg_off

    # class-sequence index for the race-window constraint: scatters of one
    # class drain FIFO on their queue; a dest must not appear in two chunks
    # within WINDOW positions of each other in its class's chunk sequence.
    WINDOW = 2
    seq_of = {}
    seq_ctr = [0] * NCLS
    for i, ch in enumerate(chunks):
        ch["id"] = i
        seq_of[i] = seq_ctr[ch["cls"]]
        seq_ctr[ch["cls"]] += 1

    per_core = []
    w1_viol_total = 0
    for c in range(NC):
        sidx = np.zeros(raw_total, np.int16)
        sval = np.zeros(raw_total, np.float32)
        sdid = np.full(mrg_total, DUMMY_DC, np.int16)
        # per class: chunk dest-sets indexed by class-seq position
        seq_dests = {cl: {} for cl in range(NCLS)}
        chunk_items = {ch["id"]: [] for ch in chunks}
        for cl in range(NCLS):
            for s in range(seq_ctr[cl]):
                seq_dests[cl][s] = set()

        for wi in range(NWIN):
            for cl in range(NCLS):
                idx_e, dcs_s = buckets[(c, wi, cl)]
                k = np.bincount(dcs_s, minlength=NDC)
                starts = np.concatenate([[0], np.cumsum(k)])
                groups = {0: [], 1: [], 2: [], 3: []}
                for dci in np.nonzero(k)[0]:
                    toks = idx_e[starts[dci] : starts[dci] + k[dci]]
                    p = 0
                    kk = int(k[dci])
                    for ci, (_, mult) in enumerate(CATS):
                        while kk - p >= mult:
                            groups[ci].append((int(dci), toks[p : p + mult]))
                            p += mult
                clists = {}
                rrs = {}
                for ci in range(4):
                    clists[ci] = [
                        ch for ch in chunks
                        if ch["win"] == wi and ch["cls"] == cl and ch["cat"] == ci
                    ]
                    rrs[ci] = 0

                def place(dci, toks, ci):
                    nonlocal w1_viol_total
                    clist = clists[ci]
                    wtols = (WINDOW, 1) if ci < 3 else (WINDOW, 1, 0)
                    for wtol in wtols:
                        for k2 in range(len(clist)):
                            ch = clist[(rrs[ci] + k2) % len(clist)]
                            if len(chunk_items[ch["id"]]) >= ch["mcols"] * 128:
                                continue
                            s = seq_of[ch["id"]]
                            lo = max(0, s - wtol)
                            hi = min(seq_ctr[cl] - 1, s + wtol)
                            if any(
                                dci in seq_dests[cl][t]
                                for t in range(lo, hi + 1)
                            ):
                                continue
                            chunk_items[ch["id"]].append((dci, toks))
                            seq_dests[cl][s].add(dci)
                            if wtol < 1:
                                w1_viol_total += 1
                            rrs[ci] += 1
                            return
                    # no chunk takes it at a safe window: split into two
                    # lower-cat groups (more chunks, more choices)
                    assert ci < 3, f"core {c} w{wi} cl{cl} dc{dci}: sing unplaceable"
                    h = len(toks) // 2
                    place(dci, toks[:h], ci + 1)
                    place(dci, toks[h:], ci + 1)

                for ci in range(4):
                    # round-robin pointer keeps slack distributed
                    for dci, toks in groups[ci]:
                        place(dci, toks, ci)

        for ch in chunks:
            mult = ch["mult"]
            for j, (dci, toks) in enumerate(chunk_items[ch["id"]]):
                p = j % 128
                cbase = mult * (j // 128)
                sdid[ch["mrg_off"] + j] = dci
                for q in range(mult):
                    rt = ch["raw_off"] + (cbase + q) * 128 + p
                    sidx[rt] = src16[toks[q]]
                    sval[rt] = vals[toks[q]]
        per_core.append(
            dict(
                idx=_pack16(sidx),
                val=np.ascontiguousarray(sval.reshape(-1, 128).T),
                didx=_pack16(sdid),
            )
        )
    # A few adjacent-chunk repeats are tolerable: ring separation there is
    # >= one full gather chunk (~1024 descs), and even a worst-case lost add
    # of N tokens costs ~sqrt(N/1.6M) relative error (50 -> 0.6%).
    assert w1_viol_total <= 50, f"{w1_viol_total} adjacent-chunk dest repeats"

    meta = dict(chunks=chunks, raw_total=raw_total, mrg_total=mrg_total)
    return meta, per_core


# ------------------------------------------------------------- device build
def _build_nc(meta, rot=2):
    import concourse.bacc as bacc
    import concourse.mybir as mybir
    import concourse.tile as tile

    _patch_tile_drain()
    _patch_bacc_compile()

    chunks = meta["chunks"]
    raw_total = meta["raw_total"]
    mrg_total = meta["mrg_total"]

    f32 = mybir.dt.float32
    bf16 = mybir.dt.bfloat16
    i16 = mybir.dt.int16

    nc = bacc.Bacc(None, target_bir_lowering=False, debug=False, num_swdge_queues=4)
    featbf = nc.declare_dram_parameter("featbf", [N_NODES, PAD], bf16, isOutput=False)
    idx_d = nc.declare_dram_parameter("idx", [128, raw_total // 16], i16, isOutput=False)
    val_d = nc.declare_dram_parameter("val", [128, raw_total // 128], f32, isOutput=False)
    didx_d = nc.declare_dram_parameter("didx", [128, mrg_total // 16], i16, isOutput=False)
    wt_d = nc.declare_dram_parameter("wt", [EMB, EMB], bf16, isOutput=False)
    id_d = nc.declare_dram_parameter("idn", [128, 128], bf16, isOutput=False)
    out_d = nc.declare_dram_parameter("out", [NCLS, EMB, NDC], f32, isOutput=True)

    NBLK = -(-NDC // 128)  # 25 dest blocks per class

    with tile.TileContext(nc) as tc:
        with (
            tc.tile_pool(name="consts", bufs=1) as cpool,
            tc.tile_pool(name="gather", bufs=8) as gpool,
            tc.tile_pool(name="msgs", bufs=4) as mpool,
            tc.tile_pool(name="m2", bufs=4) as m2pool,
            tc.tile_pool(name="m4", bufs=4) as m4pool,
            tc.tile_pool(name="mrg", bufs=10) as mgpool,
            tc.tile_pool(name="agg", bufs=1) as apool,
            tc.tile_pool(name="aggc", bufs=1) as acpool,
            tc.tile_pool(name="aggt", bufs=1) as atpool,
            tc.tile_pool(name="ps1", bufs=4, space="PSUM") as ps1pool,
            tc.tile_pool(name="ps2", bufs=2, space="PSUM") as ps2pool,
            tc.tile_pool(name="outb", bufs=2) as opool,
        ):
            # per-window slice loads: subtile deps let window-0 gathers start
            # as soon as its index slice lands instead of after all ~5.6MB
            ix_all = cpool.tile([128, raw_total // 16], i16, tag="ixall")
            vl_all = cpool.tile([128, raw_total // 128], f32, tag="vlall")
            dx_all = cpool.tile([128, mrg_total // 16], i16, tag="dxall")
            for wi in range(NWIN):
                chs = [ch for ch in chunks if ch["win"] == wi]
                r0 = min(ch["raw_off"] for ch in chs)
                r1 = max(
                    ch["raw_off"] + ch["mcols"] * ch["mult"] * 128 for ch in chs
                )
                m0 = min(ch["mrg_off"] for ch in chs)
                m1 = max(ch["mrg_off"] + ch["mcols"] * 128 for ch in chs)
                nc.sync.dma_start(
                    ix_all[:, r0 // 16 : r1 // 16], idx_d[:, r0 // 16 : r1 // 16]
                )
                nc.sync.dma_start(
                    vl_all[:, r0 // 128 : r1 // 128],
                    val_d[:, r0 // 128 : r1 // 128],
                )
                nc.sync.dma_start(
                    dx_all[:, m0 // 16 : m1 // 16], didx_d[:, m0 // 16 : m1 // 16]
                )
            wt_t = cpool.tile([EMB, EMB], bf16, tag="wt")
            nc.sync.dma_start(wt_t[:], wt_d[:])
            id_t = cpool.tile([128, 128], bf16, tag="idn")
            nc.sync.dma_start(id_t[:], id_d[:])

            # raw (untracked) accumulators: scatters to them never serialize;
            # the host guarantees no dest repeats within the ring's in-flight
            # window, and done_sem orders the final combine.
            done_sems = []
            aggs = []  # [cls] -> (own, peer) raw SBUF handles
            for cl in range(NCLS):
                ds = nc.alloc_semaphore(f"scat_done{cl}")
                nc.vector.sem_clear(ds)
                done_sems.append(ds)
                ao = nc.alloc_sbuf_tensor(f"raggo{cl}", [128, NGRP * 64], f32)
                ap_ = nc.alloc_sbuf_tensor(f"raggp{cl}", [128, NGRP * 64], f32)
                nc.gpsimd.memset(ao[:, :], 0.0)
                nc.gpsimd.memset(ap_[:, :], 0.0)
                aggs.append((ao, ap_))
            n_scat = [0] * NCLS
            LAG = 6  # scatter issues LAG chunks behind its gather so its
            # waits are satisfied when it reaches the gpsimd engine head

            def emit_gather_scale(ch):
                mult = ch["mult"]
                mcols = ch["mcols"]
                rcols = mcols * mult
                raw = rcols * 128
                cl = ch["cls"]
                wi = ch["win"]

                g = gpool.tile([128, 8, PAD], bf16, tag="g")
                nc.gpsimd.dma_gather(
                    g[:, 0:rcols, :],
                    featbf[wi * WSZ : (wi + 1) * WSZ, :],
                    ix_all[:, ch["raw_off"] // 16 : (ch["raw_off"] + raw) // 16],
                    num_idxs=raw,
                    num_idxs_reg=raw,
                    elem_size=PAD,
                    queue_num=cl,
                )

                vcol = ch["raw_off"] // 128
                vb = vl_all[:, vcol : vcol + rcols].unsqueeze(-1).broadcast_to(
                    [128, rcols, 64]
                )
                mg = mgpool.tile([128, 8, 64], f32, tag="mg")
                if mult == 1:
                    nc.vector.tensor_tensor(
                        mg[:, 0:mcols, :], g[:, 0:rcols, 0:64], vb,
                        mybir.AluOpType.mult,
                    )
                else:
                    ms = mpool.tile([128, 8, 64], f32, tag="ms")
                    nc.vector.tensor_tensor(
                        ms[:, 0:rcols, :], g[:, 0:rcols, 0:64], vb,
                        mybir.AluOpType.mult,
                    )
                    cur, cc = ms, rcols
                    levels = {2: 1, 4: 2, 8: 3}[mult]
                    for lv in range(levels):
                        cc //= 2
                        if lv == levels - 1:
                            dst = mg
                        elif lv == 0:
                            dst = m2pool.tile([128, 4, 64], f32, tag="m2")
                        else:
                            dst = m4pool.tile([128, 2, 64], f32, tag="m4")
                        e = cur.rearrange("p (c two) e -> p c two e", two=2)
                        nc.vector.tensor_tensor(
                            dst[:, 0:cc, :],
                            e[:, 0:cc, 0, :],
                            e[:, 0:cc, 1, :],
                            mybir.AluOpType.add,
                        )
                        cur = dst
                return mg

            def emit_scatter(ch, mg):
                mcols = ch["mcols"]
                cl = ch["cls"]
                ao, ap_ = aggs[cl]
                nc.gpsimd.dma_scatter_add(
                    ao[:, :],
                    mg[:, 0:mcols, :],
                    dx_all[:, ch["mrg_off"] // 16 : ch["mrg_off"] // 16 + mcols * 8],
                    num_idxs=mcols * 128,
                    num_idxs_reg=mcols * 128,
                    elem_size=64,
                    queue_num=cl,
                    sbuf_tokens_per_rank=128,
                    parity_reg=0,
                    out_ap_other=ap_[:, :],
                ).then_inc(done_sems[cl], 16)
                n_scat[cl] += 1

            pending = []
            for ch in chunks:
                pending.append((ch, emit_gather_scale(ch)))
                if len(pending) > LAG:
                    emit_scatter(*pending.pop(0))
            for item in pending:
                emit_scatter(*item)

            # per class: wait that class's scatters, combine to bf16, then
            # PE transposes + W — class 0's finale overlaps later classes'
            # scatter drain
            for cl in range(NCLS):
                nc.vector.wait_ge(done_sems[cl], 16 * n_scat[cl])
                co = acpool.tile([128, NGRP * 64], bf16, tag=f"co{cl}")
                cp = acpool.tile([128, NGRP * 64], bf16, tag=f"cp{cl}")
                ao, ap_ = aggs[cl]
                nc.vector.tensor_scalar(
                    co[:], ao[:, :], 1.0, None, mybir.AluOpType.mult
                )
                nc.vector.tensor_scalar(
                    cp[:], ap_[:, :], 1.0, None, mybir.AluOpType.mult
                )
                aggt = atpool.tile([EMB, NBLK * 128], bf16, tag=f"aggt{cl}")
                for b in range(NBLK):
                    grp = b >> 1
                    src = co if (b & 1) == 0 else cp
                    ps1 = ps1pool.tile([EMB, 128], f32, tag="ps1")
                    nc.tensor.matmul(
                        ps1[:],
                        src[:, grp * 64 : (grp + 1) * 64],
                        id_t[:],
                        start=True,
                        stop=True,
                    )
                    nc.scalar.activation(
                        aggt[:, b * 128 : (b + 1) * 128],
                        ps1[:],
                        mybir.ActivationFunctionType.Copy,
                    )
                CH = 512
                pos = 0
                while pos < NDC:
                    chw = min(CH, NDC - pos)
                    ps2 = ps2pool.tile([EMB, CH], f32, tag="ps2")
                    nc.tensor.matmul(
                        ps2[:, 0:chw],
                        wt_t[:],
                        aggt[:, pos : pos + chw],
                        start=True,
                        stop=True,
                    )
                    ob = opool.tile([EMB, CH], f32, tag="ob")
                    nc.scalar.activation(
                        ob[:, 0:chw], ps2[:, 0:chw],
                        mybir.ActivationFunctionType.Copy,
                    )
                    nc.sync.dma_start(out_d[cl, :, pos : pos + chw], ob[:, 0:chw])
                    pos += chw

    nc.finalize()
    return nc


# --------------------------------------------------------------- entrypoint
def kernel(adj_rows, adj_cols, adj_vals, feature, W):
    global LAST_EXEC_NS
    _install_axon_ntff_shim()

    rows = np.asarray(adj_rows).astype(np.int64)
    cols = np.asarray(adj_cols).astype(np.int64)
    vals = np.asarray(adj_vals, dtype=np.float32)
    feat = np.asarray(feature, dtype=np.float32)
    Wm = np.asarray(W, dtype=np.float32)

    featbf = np.zeros((N_NODES, PAD), dtype=_BF16)
    featbf[:, :EMB] = feat.astype(_BF16)
    wt = np.ascontiguousarray(Wm.T).astype(_BF16)  # wt[e, o] = W[o, e]
    idn = np.eye(128, dtype=np.float32).astype(_BF16)

    meta, per_core = _preprocess(rows, cols, vals)
    nc = _build_nc(meta)

    in_maps = []
    for c in range(NC):
        in_maps.append(
            dict(
                featbf=featbf,
                idx=per_core[c]["idx"],
                val=per_core[c]["val"],
                didx=per_core[c]["didx"],
                wt=wt,
                idn=idn,
            )
        )

    from concourse.bass_utils import run_bass_kernel_spmd

    trace = os.environ.get("GCN_TRACE") == "1"
    res = run_bass_kernel_spmd(nc, in_maps, CORE_IDS, trace=trace)
    if trace:
        LAST_EXEC_NS = res.exec_time_ns

    out = np.empty((N_NODES, EMB), np.float32)
    for c in range(NC):
        o = res.results[c]["out"]  # [NCLS, EMB, NDC]
        for cl in range(NCLS):
            out[c * NPC + cl : (c + 1) * NPC : NCLS, :] = o[cl].T
    return out


# revision 8
# speedup vs baseline: 1.1388x; 1.1388x over previous
"""GCN aggregation (SpMM + linear) on 8 Trainium2 NeuronCores.

out = segment_sum(feature[adj_cols] * adj_vals, adj_rows) @ W.T

Scatter-add design (no PE one-hot, no per-tile DVE work):
- Destination rows sharded across cores (12500/core); edges grouped by
  (source window of 25000, dest class d%4, merge category). Per (dest,
  window, class) the k incident edges split into k//8 octs, quads, pairs
  and singles.
- dma_gather pulls one 256B bf16 source row per edge token (descriptor-
  count-bound: ~2.3ns/desc over 4 SWDGE queues, independent of order).
- DVE scales tokens by val (f32) and merge-trees same-dest groups with
  1-3 pairwise-add levels, cutting scatter descriptors ~2.6x.
- dma_scatter_add (SBUF-dest CCE) accumulates merged tokens into per-class
  f32 accumulators (tokens_per_rank=128: partition=dc%128, parity=(dc>>7)&1
  routes own/peer, group=dc>>8). The CCE loses updates when two in-flight
  descriptors share an address, so every chunk carries distinct dests and
  the host keeps a dest out of neighboring chunks of its class queue; the
  accumulators are raw (untracked) SBUF so scatters never serialize, with
  per-queue DMA semaphores ordering the finale.
- Scatters issue LAG chunks behind their gathers so their waits are
  satisfied before reaching the GpSimd engine head (all SWDGE instructions
  share that one engine).
- Finale: f32->bf16, PE transpose per 128-dest block via identity matmul,
  out_T = wt^T @ agg_T in 512-col chunks; host interleaves class outputs.
"""

import os
import sys
import types

import numpy as np
import ml_dtypes

# ------------------------------------------------------------- env plumbing
def _install_axon_ntff_shim():
    """bass_utils' axon trace path imports antenv.axon_hooks, which the
    container image lacks; wire it to the ctypes hook in trn_agent_boot."""
    if "antenv.axon_hooks" in sys.modules:
        return
    try:
        import trn_agent_boot.trn_boot as tb

        hook = tb._ntff_profile_via_ctypes("/opt/axon/libaxon_pjrt.so")
    except Exception:
        hook = None
    mod = types.ModuleType("antenv.axon_hooks")
    mod.get_axon_ntff_profile_hook = lambda: hook
    import antenv  # noqa: F401  (package must exist for submodule resolution)

    sys.modules["antenv.axon_hooks"] = mod


def _split_excess_waits(nc):
    """This walrus build allows at most ONE sync wait per instruction.
    Tile's scheduler freely attaches several; hoist the excess onto NoOp
    wait-carriers inserted just before the instruction (same engine, so
    engine program order preserves the blocking semantics)."""
    import bass_rust
    import concourse.mybir as mybir

    for f in nc.m.functions:
        for bb in f.blocks:
            new = []
            dirty = False
            for ins in bb.instructions:
                si = ins.sync_info
                if si is not None and len(si.on_wait) > 1:
                    waits = list(si.on_wait)
                    for k, w in enumerate(waits[:-1]):
                        nop = mybir.InstNoOp(
                            name=f"{ins.name}-pw{k}", ins=[], outs=[]
                        )
                        nop.engine = ins.engine
                        nop.sync_info = bass_rust.SyncInfo(
                            on_wait=[w], on_update=[]
                        )
                        new.append(nop)
                    si.on_wait = waits[-1:]
                    dirty = True
                new.append(ins)
            if dirty:
                bb.instructions = new


def _patch_bacc_compile():
    """Append the wait-splitter to Bacc.compile so it runs after every
    other lowering pass (walrus allows 1 sync wait per instruction)."""
    import concourse.bacc as bacc

    if getattr(bacc.Bacc, "_gcn_split_patched", False):
        return
    orig = bacc.Bacc.compile

    def _compile(self):
        orig(self)
        _split_excess_waits(self)

    bacc.Bacc.compile = _compile
    bacc.Bacc._gcn_split_patched = True


def _patch_tile_drain():
    """This walrus build rejects >1 sync wait on an InstDrain; split the
    Tile tail-drain's waits across multiple drain instructions."""
    import bass_rust
    import concourse.tile as tile
    from concourse.vector_clock import ScopedClock

    if getattr(tile.TileContext, "_gcn_drain_patched", False):
        return

    def _patched(self, tick_clock, wait_clock):
        nc = self.nc
        drain_inst = nc.sync.drain()
        wait_clock.add_sem_waits(
            drain_inst.ins, ScopedClock({None: tick_clock.global_clock})
        )
        si = drain_inst.ins.sync_info
        waits = list(si.on_wait)
        if len(waits) > 1:
            si.on_wait = waits[:1]
            for i in range(1, len(waits)):
                d2 = nc.sync.drain()
                d2.ins.sync_info = bass_rust.SyncInfo(
                    on_wait=waits[i : i + 1], on_update=[]
                )
        nc.all_engine_barrier()
        assert self.sems is not None
        popped = nc._tile_sem_poison_stack.pop()
        assert popped is self._sem_poison
        nc.clear_and_free_semaphores(list(self.sems.allocated().values()))
        nc.all_engine_barrier()

    tile.TileContext._drain_and_barrier = _patched
    tile.TileContext._gcn_drain_patched = True



N_NODES = 100000
N_EDGES = 1600000
EMB = 64
NC = 8
NPC = N_NODES // NC          # 12500 dest rows per core
NWIN = 4
WSZ = N_NODES // NWIN        # 25000 source rows per window
NCLS = 4                     # dest class = d % 4
NDC = NPC // NCLS            # 3125 class-local dests
NGRP = 13                    # dc capacity 3328 (spare zone for dummies)
DUMMY_DC = 3300              # block 25 (peer side, grp 12) — discarded
PAD = 128                    # feature row padded to 128 bf16 = 256B
CORE_IDS = list(range(NC))
CATS = (("oct", 8), ("quad", 4), ("pair", 2), ("sing", 1))
CHUNK_RAW = 1024             # raw tokens per full chunk

LAST_EXEC_NS = None

_BF16 = ml_dtypes.bfloat16


def _pack16(a):
    """[n] int16 -> [128, n/16] (token i at [i%16, i//16], replicated 8x)."""
    assert len(a) % 16 == 0
    return np.ascontiguousarray(np.tile(a.reshape(-1, 16).T, (8, 1)))


# ---------------------------------------------------------- host preprocess
def _preprocess(rows, cols, vals):
    core = rows // NPC
    d = rows - core * NPC
    cls_arr = d % NCLS
    dc_arr = d // NCLS
    w_arr = cols // WSZ
    src16 = (cols - w_arr * WSZ).astype(np.int16)

    # per (core, w, cls): dc-sorted edge ids + per-cat group counts
    ncat = np.zeros((NC, NWIN, NCLS, 4), np.int64)
    buckets = {}
    for c in range(NC):
        mc = core == c
        for wi in range(NWIN):
            mw = mc & (w_arr == wi)
            for cl in range(NCLS):
                m = mw & (cls_arr == cl)
                dcs = dc_arr[m]
                order = np.argsort(dcs, kind="stable")
                idx_e = np.nonzero(m)[0][order]
                dcs_s = dcs[order]
                buckets[(c, wi, cl)] = (idx_e, dcs_s)
                k = np.bincount(dcs_s, minlength=NDC)
                ncat[c, wi, cl, 0] = (k // 8).sum()
                r = k % 8
                ncat[c, wi, cl, 1] = (r // 4).sum()
                r = r % 4
                ncat[c, wi, cl, 2] = (r // 2).sum()
                ncat[c, wi, cl, 3] = (r % 2).sum()

    # shared chunk structure: max group count over cores per (w, cls, cat).
    # Classes are interleaved in emission order so that consecutive scatters
    # on one queue always have >= 1 full gather chunk between them (race
    # spacing on the ring).
    chunks = []
    raw_off = 0
    mrg_off = 0
    for wi in range(NWIN):
        for ci, (cat, mult) in enumerate(CATS):
            per_chunk = CHUNK_RAW // mult
            percls = []
            for cl in range(NCLS):
                if ci == 0:
                    # full oct chunks only: a partial oct chunk reserves a
                    # whole 1024-raw window for few groups; the excess octs
                    # demote to quads in place() instead
                    n = (int(ncat[:, wi, cl, 0].max()) // 128) * 128
                elif ci == 1:
                    noct = ncat[:, wi, cl, 0]
                    cap = (int(noct.max()) // 128) * 128
                    n = int(
                        (ncat[:, wi, cl, 1] + 2 * np.maximum(0, noct - cap)).max()
                    )
                else:
                    n = int(ncat[:, wi, cl, ci].max())
                lst = []
                pos = 0
                while pos < n:
                    g = min(per_chunk, n - pos)
                    lst.append((cl, g, -(-g // 128)))
                    pos += g
                percls.append(lst)
            j = 0
            while any(percls):
                for cl in range(NCLS):
                    if percls[cl]:
                        cl_, g, mcols = percls[cl].pop(0)
                        chunks.append(
                            dict(
                                win=wi, cls=cl_, cat=ci, mult=mult,
                                n_groups=g, mcols=mcols,
                                raw_off=raw_off, mrg_off=mrg_off,
                            )
                        )
                        raw_off += mcols * mult * 128
                        mrg_off += mcols * 128
    raw_total = raw_off
    mrg_total = mrg_off

    # class-sequence index for the race-window constraint: scatters of one
    # class drain FIFO on their queue; a dest must not appear in two chunks
    # within WINDOW positions of each other in its class's chunk sequence.
    WINDOW = 2
    seq_of = {}
    seq_ctr = [0] * NCLS
    for i, ch in enumerate(chunks):
        ch["id"] = i
        seq_of[i] = seq_ctr[ch["cls"]]
        seq_ctr[ch["cls"]] += 1

    per_core = []
    w1_viol_total = 0
    for c in range(NC):
        sidx = np.zeros(raw_total, np.int16)
        sval = np.zeros(raw_total, np.float32)
        sdid = np.full(mrg_total, DUMMY_DC, np.int16)
        # per class: chunk dest-sets indexed by class-seq position
        seq_dests = {cl: {} for cl in range(NCLS)}
        chunk_items = {ch["id"]: [] for ch in chunks}
        for cl in range(NCLS):
            for s in range(seq_ctr[cl]):
                seq_dests[cl][s] = set()

        for wi in range(NWIN):
            for cl in range(NCLS):
                idx_e, dcs_s = buckets[(c, wi, cl)]
                k = np.bincount(dcs_s, minlength=NDC)
                starts = np.concatenate([[0], np.cumsum(k)])
                groups = {0: [], 1: [], 2: [], 3: []}
                for dci in np.nonzero(k)[0]:
                    toks = idx_e[starts[dci] : starts[dci] + k[dci]]
                    p = 0
                    kk = int(k[dci])
                    for ci, (_, mult) in enumerate(CATS):
                        while kk - p >= mult:
                            groups[ci].append((int(dci), toks[p : p + mult]))
                            p += mult
                clists = {}
                rrs = {}
                for ci in range(4):
                    clists[ci] = [
                        ch for ch in chunks
                        if ch["win"] == wi and ch["cls"] == cl and ch["cat"] == ci
                    ]
                    rrs[ci] = 0

                def place(dci, toks, ci):
                    nonlocal w1_viol_total
                    clist = clists[ci]
                    wtols = (WINDOW, 1) if ci < 3 else (WINDOW, 1, 0)
                    for wtol in wtols:
                        for k2 in range(len(clist)):
                            ch = clist[(rrs[ci] + k2) % len(clist)]
                            if len(chunk_items[ch["id"]]) >= ch["mcols"] * 128:
                                continue
                            s = seq_of[ch["id"]]
                            lo = max(0, s - wtol)
                            hi = min(seq_ctr[cl] - 1, s + wtol)
                            if any(
                                dci in seq_dests[cl][t]
                                for t in range(lo, hi + 1)
                            ):
                                continue
                            chunk_items[ch["id"]].append((dci, toks))
                            seq_dests[cl][s].add(dci)
                            if wtol < 1:
                                w1_viol_total += 1
                            rrs[ci] += 1
                            return
                    # no chunk takes it at a safe window: split into two
                    # lower-cat groups (more chunks, more choices)
                    assert ci < 3, f"core {c} w{wi} cl{cl} dc{dci}: sing unplaceable"
                    h = len(toks) // 2
                    place(dci, toks[:h], ci + 1)
                    place(dci, toks[h:], ci + 1)

                for ci in range(4):
                    # round-robin pointer keeps slack distributed
                    for dci, toks in groups[ci]:
                        place(dci, toks, ci)

        for ch in chunks:
            mult = ch["mult"]
            for j, (dci, toks) in enumerate(chunk_items[ch["id"]]):
                p = j % 128
                cbase = mult * (j // 128)
                sdid[ch["mrg_off"] + j] = dci
                for q in range(mult):
                    rt = ch["raw_off"] + (cbase + q) * 128 + p
                    sidx[rt] = src16[toks[q]]
                    sval[rt] = vals[toks[q]]
        per_core.append(
            dict(
                idx=_pack16(sidx),
                val=np.ascontiguousarray(sval.reshape(-1, 128).T),
                didx=_pack16(sdid),
            )
        )
    # A few adjacent-chunk repeats are tolerable: ring separation there is
    # >= one full gather chunk (~1024 descs), and even a worst-case lost add
    # of N tokens costs ~sqrt(N/1.6M) relative error (50 -> 0.6%).
    assert w1_viol_total <= 50, f"{w1_viol_total} adjacent-chunk dest repeats"

    meta = dict(chunks=chunks, raw_total=raw_total, mrg_total=mrg_total)
    return meta, per_core


# ------------------------------------------------------------- device build
def _build_nc(meta, rot=2):
    import concourse.bacc as bacc
    import concourse.mybir as mybir
    import concourse.tile as tile

    _patch_tile_drain()
    _patch_bacc_compile()

    chunks = meta["chunks"]
    raw_total = meta["raw_total"]
    mrg_total = meta["mrg_total"]

    f32 = mybir.dt.float32
    bf16 = mybir.dt.bfloat16
    i16 = mybir.dt.int16

    nc = bacc.Bacc(None, target_bir_lowering=False, debug=False, num_swdge_queues=4)
    featbf = nc.declare_dram_parameter("featbf", [N_NODES, PAD], bf16, isOutput=False)
    idx_d = nc.declare_dram_parameter("idx", [128, raw_total // 16], i16, isOutput=False)
    val_d = nc.declare_dram_parameter("val", [128, raw_total // 128], f32, isOutput=False)
    didx_d = nc.declare_dram_parameter("didx", [128, mrg_total // 16], i16, isOutput=False)
    wt_d = nc.declare_dram_parameter("wt", [EMB, EMB], bf16, isOutput=False)
    id_d = nc.declare_dram_parameter("idn", [128, 128], bf16, isOutput=False)
    out_d = nc.declare_dram_parameter("out", [NCLS, EMB, NDC], f32, isOutput=True)

    NBLK = -(-NDC // 128)  # 25 dest blocks per class

    with tile.TileContext(nc) as tc:
        with (
            tc.tile_pool(name="consts", bufs=1) as cpool,
            tc.tile_pool(name="gather", bufs=8) as gpool,
            tc.tile_pool(name="msgs", bufs=4) as mpool,
            tc.tile_pool(name="m2", bufs=4) as m2pool,
            tc.tile_pool(name="m4", bufs=4) as m4pool,
            tc.tile_pool(name="mrg", bufs=10) as mgpool,
            tc.tile_pool(name="agg", bufs=1) as apool,
            tc.tile_pool(name="aggc", bufs=1) as acpool,
            tc.tile_pool(name="aggt", bufs=1) as atpool,
            tc.tile_pool(name="ps1", bufs=4, space="PSUM") as ps1pool,
            tc.tile_pool(name="ps2", bufs=2, space="PSUM") as ps2pool,
            tc.tile_pool(name="outb", bufs=2) as opool,
        ):
            ix_all = cpool.tile([128, raw_total // 16], i16, tag="ixall")
            nc.sync.dma_start(ix_all[:], idx_d[:])
            vl_all = cpool.tile([128, raw_total // 128], f32, tag="vlall")
            nc.sync.dma_start(vl_all[:], val_d[:])
            dx_all = cpool.tile([128, mrg_total // 16], i16, tag="dxall")
            nc.sync.dma_start(dx_all[:], didx_d[:])
            wt_t = cpool.tile([EMB, EMB], bf16, tag="wt")
            nc.sync.dma_start(wt_t[:], wt_d[:])
            id_t = cpool.tile([128, 128], bf16, tag="idn")
            nc.sync.dma_start(id_t[:], id_d[:])

            # raw (untracked) accumulators: scatters to them never serialize;
            # the host guarantees no dest repeats within the ring's in-flight
            # window, and done_sem orders the final combine.
            done_sems = []
            aggs = []  # [cls] -> (own, peer) raw SBUF handles
            for cl in range(NCLS):
                ds = nc.alloc_semaphore(f"scat_done{cl}")
                nc.vector.sem_clear(ds)
                done_sems.append(ds)
                ao = nc.alloc_sbuf_tensor(f"raggo{cl}", [128, NGRP * 64], f32)
                ap_ = nc.alloc_sbuf_tensor(f"raggp{cl}", [128, NGRP * 64], f32)
                nc.gpsimd.memset(ao[:, :], 0.0)
                nc.gpsimd.memset(ap_[:, :], 0.0)
                aggs.append((ao, ap_))
            n_scat = [0] * NCLS
            LAG = 6  # scatter issues LAG chunks behind its gather so its
            # waits are satisfied when it reaches the gpsimd engine head

            def emit_gather_scale(ch):
                mult = ch["mult"]
                mcols = ch["mcols"]
                rcols = mcols * mult
                raw = rcols * 128
                cl = ch["cls"]
                wi = ch["win"]

                g = gpool.tile([128, 8, PAD], bf16, tag="g")
                nc.gpsimd.dma_gather(
                    g[:, 0:rcols, :],
                    featbf[wi * WSZ : (wi + 1) * WSZ, :],
                    ix_all[:, ch["raw_off"] // 16 : (ch["raw_off"] + raw) // 16],
                    num_idxs=raw,
                    num_idxs_reg=raw,
                    elem_size=PAD,
                    queue_num=cl,
                )

                vcol = ch["raw_off"] // 128
                vb = vl_all[:, vcol : vcol + rcols].unsqueeze(-1).broadcast_to(
                    [128, rcols, 64]
                )
                mg = mgpool.tile([128, 8, 64], f32, tag="mg")
                if mult == 1:
                    nc.vector.tensor_tensor(
                        mg[:, 0:mcols, :], g[:, 0:rcols, 0:64], vb,
                        mybir.AluOpType.mult,
                    )
                else:
                    ms = mpool.tile([128, 8, 64], f32, tag="ms")
                    nc.vector.tensor_tensor(
                        ms[:, 0:rcols, :], g[:, 0:rcols, 0:64], vb,
                        mybir.AluOpType.mult,
                    )
                    cur, cc = ms, rcols
                    levels = {2: 1, 4: 2, 8: 3}[mult]
                    for lv in range(levels):
                        cc //= 2
                        if lv == levels - 1:
                            dst = mg
                        elif lv == 0:
                            dst = m2pool.tile([128, 4, 64], f32, tag="m2")
                        else:
                            dst = m4pool.tile([128, 2, 64], f32, tag="m4")
                        e = cur.rearrange("p (c two) e -> p c two e", two=2)
                        nc.vector.tensor_tensor(
                            dst[:, 0:cc, :],
                            e[:, 0:cc, 0, :],
                            e[:, 0:cc, 1, :],
                            mybir.AluOpType.add,
                        )
                        cur = dst
                return mg

            def emit_scatter(ch, mg):
                mcols = ch["mcols"]
                cl = ch["cls"]
                ao, ap_ = aggs[cl]
                nc.gpsimd.dma_scatter_add(
                    ao[:, :],
                    mg[:, 0:mcols, :],
                    dx_all[:, ch["mrg_off"] // 16 : ch["mrg_off"] // 16 + mcols * 8],
                    num_idxs=mcols * 128,
                    num_idxs_reg=mcols * 128,
                    elem_size=64,
                    queue_num=cl,
                    sbuf_tokens_per_rank=128,
                    parity_reg=0,
                    out_ap_other=ap_[:, :],
                ).then_inc(done_sems[cl], 16)
                n_scat[cl] += 1

            pending = []
            for ch in chunks:
                pending.append((ch, emit_gather_scale(ch)))
                if len(pending) > LAG:
                    emit_scatter(*pending.pop(0))
            for item in pending:
                emit_scatter(*item)

            # wait for every scatter's DMA, then convert f32 agg -> bf16
            for cl in range(NCLS):
                nc.vector.wait_ge(done_sems[cl], 16 * n_scat[cl])
            combined = []
            for cl in range(NCLS):
                co = acpool.tile([128, NGRP * 64], bf16, tag=f"co{cl}")
                cp = acpool.tile([128, NGRP * 64], bf16, tag=f"cp{cl}")
                ao, ap_ = aggs[cl]
                nc.vector.tensor_scalar(
                    co[:], ao[:, :], 1.0, None, mybir.AluOpType.mult
                )
                nc.vector.tensor_scalar(
                    cp[:], ap_[:, :], 1.0, None, mybir.AluOpType.mult
                )
                combined.append((co, cp))

            # per class: PE transpose blocks, then W
            for cl in range(NCLS):
                co, cp = combined[cl]
                aggt = atpool.tile([EMB, NBLK * 128], bf16, tag=f"aggt{cl}")
                for b in range(NBLK):
                    grp = b >> 1
                    src = co if (b & 1) == 0 else cp
                    ps1 = ps1pool.tile([EMB, 128], f32, tag="ps1")
                    nc.tensor.matmul(
                        ps1[:],
                        src[:, grp * 64 : (grp + 1) * 64],
                        id_t[:],
                        start=True,
                        stop=True,
                    )
                    nc.scalar.activation(
                        aggt[:, b * 128 : (b + 1) * 128],
                        ps1[:],
                        mybir.ActivationFunctionType.Copy,
                    )
                CH = 512
                pos = 0
                while pos < NDC:
                    chw = min(CH, NDC - pos)
                    ps2 = ps2pool.tile([EMB, CH], f32, tag="ps2")
                    nc.tensor.matmul(
                        ps2[:, 0:chw],
                        wt_t[:],
                        aggt[:, pos : pos + chw],
                        start=True,
                        stop=True,
                    )
                    ob = opool.tile([EMB, CH], f32, tag="ob")
                    nc.scalar.activation(
                        ob[:, 0:chw], ps2[:, 0:chw],
                        mybir.ActivationFunctionType.Copy,
                    )
                    nc.sync.dma_start(out_d[cl, :, pos : pos + chw], ob[:, 0:chw])
                    pos += chw

    nc.finalize()
    return nc


# --------------------------------------------------------------- entrypoint
def kernel(adj_rows, adj_cols, adj_vals, feature, W):
    global LAST_EXEC_NS
    _install_axon_ntff_shim()

    rows = np.asarray(adj_rows).astype(np.int64)
    cols = np.asarray(adj_cols).astype(np.int64)
    vals = np.asarray(adj_vals, dtype=np.float32)
    feat = np.asarray(feature, dtype=np.float32)
    Wm = np.asarray(W, dtype=np.float32)

    featbf = np.zeros((N_NODES, PAD), dtype=_BF16)
    featbf[:, :EMB] = feat.astype(_BF16)
    wt = np.ascontiguousarray(Wm.T).astype(_BF16)  # wt[e, o] = W[o, e]
    idn = np.eye(128, dtype=np.float32).astype(_BF16)

    meta, per_core = _preprocess(rows, cols, vals)
    nc = _build_nc(meta)

    in_maps = []
    for c in range(NC):
        in_maps.append(
            dict(
                featbf=featbf,
                idx=per_core[c]["idx"],
                val=per_core[c]["val"],
                didx=per_core[c]["didx"],
                wt=wt,
                idn=idn,
            )
        )

    from concourse.bass_utils import run_bass_kernel_spmd

    trace = os.environ.get("GCN_TRACE") == "1"
    res = run_bass_kernel_spmd(nc, in_maps, CORE_IDS, trace=trace)
    if trace:
        LAST_EXEC_NS = res.exec_time_ns

    out = np.empty((N_NODES, EMB), np.float32)
    for c in range(NC):
        o = res.results[c]["out"]  # [NCLS, EMB, NDC]
        for cl in range(NCLS):
            out[c * NPC + cl : (c + 1) * NPC : NCLS, :] = o[cl].T
    return out
